# revision 18
# baseline (speedup 1.0000x reference)
"""MoE layer (shared expert + 8 routed experts, top-2) on 8 TRN2 NeuronCores.

Strategy: token-data-parallel. Each core owns 1/8 of the B*S=8192 tokens and
holds ALL expert weights in its HBM (host replicates them — host->HBM staging
is not part of device exec time). On device, per core:
  1. Router: logits = x @ gw^T, softmax-free top-2 combine weights
     (cw_e = exp(l_e - m1) / (exp(0) + exp(m2 - m1) + 1e-8 * sum_exp)).
  2. For shared + 8 experts: SwiGLU with fp32r matmuls
     (stage 1: H^T = Wgu @ X^T tokens-on-free; silu(G)*U -> A^T;
      stage 2: OUT = A^T.T @ Wdn^T tokens-on-partitions), combined into an
     fp32 accumulator with per-token scalar weights.
No collectives needed; output slices concatenate on host.
"""

import numpy as np
from contextlib import ExitStack

import concourse.bass as bass
import concourse.mybir as mybir
import concourse.tile as tile
from concourse import bacc
from concourse.bass_utils import run_bass_kernel_spmd

# Problem shape (hardcoded per contract)
B, S, D = 4, 2048, 1024
E = 8           # routed experts
I = 938         # expert intermediate dim
IP = 1024       # padded intermediate (8 * 128)
GU = 2 * IP     # padded gate+up columns
NE = E + 1      # shared + routed
N_CORES = 8
T = (B * S) // N_CORES  # tokens per core = 1024

P = 128
KD = D // P      # k-chunks over D
KI = IP // P     # k-chunks over padded I
MT = T // P      # token chunks of 128
NT2 = T // 512   # token chunks of 512
ND2 = D // 512   # output D chunks of 512

F32 = mybir.dt.float32
F32R = mybir.dt.float32r
AF = mybir.ActivationFunctionType
OP = mybir.AluOpType


def build_moe():
    nc = bacc.Bacc("TRN2", target_bir_lowering=False, debug=False,
                   enable_asserts=True, num_devices=N_CORES)
    xT = nc.dram_tensor("xT", [D, T], F32R, kind="ExternalInput")
    gwT = nc.dram_tensor("gwT", [D, E], F32R, kind="ExternalInput")
    wgu = nc.dram_tensor("wgu", [NE, D, GU], F32R, kind="ExternalInput")
    wdn = nc.dram_tensor("wdn", [NE, IP, D], F32R, kind="ExternalInput")
    out = nc.dram_tensor("out", [T, D], F32, kind="ExternalOutput")

    def r(ap):
        return ap

    with tile.TileContext(nc) as tc, ExitStack() as ctx:
        xt_pool = ctx.enter_context(tc.tile_pool(name="xt", bufs=KD))
        wgu_pool = ctx.enter_context(tc.tile_pool(name="wgu", bufs=8))
        wdn_pool = ctx.enter_context(tc.tile_pool(name="wdn", bufs=8))
        a_pool = ctx.enter_context(tc.tile_pool(name="a", bufs=8))
        acc_pool = ctx.enter_context(tc.tile_pool(name="acc", bufs=MT))
        tmp_pool = ctx.enter_context(tc.tile_pool(name="tmp", bufs=2))
        rt_pool = ctx.enter_context(tc.tile_pool(name="rt", bufs=MT))
        rb_pool = ctx.enter_context(tc.tile_pool(name="rb", bufs=2))
        ps_g = ctx.enter_context(tc.tile_pool(name="psg", bufs=2, space="PSUM"))
        ps_u = ctx.enter_context(tc.tile_pool(name="psu", bufs=2, space="PSUM"))
        ps_o = ctx.enter_context(tc.tile_pool(name="pso", bufs=3, space="PSUM"))

        # ---- load X^T (resident across all experts) ----
        xts = []
        for k in range(KD):
            t = xt_pool.tile([P, T], F32R, tag="xt")
            nc.sync.dma_start(t[:], xT[k * P:(k + 1) * P, :])
            xts.append(t)

        # ---- router ----
        gwts = []
        for k in range(KD):
            g = rt_pool.tile([P, E], F32R, tag="gw", name=f"gw{k}")
            nc.sync.dma_start(g[:], gwT[k * P:(k + 1) * P, :])
            gwts.append(g)

        cw_tiles = []
        for mt in range(MT):
            pl = ps_o.tile([P, E], F32, tag="pso", name=f"pl{mt}")
            for k in range(KD):
                nc.tensor.matmul(pl[:], xts[k][:, mt * P:(mt + 1) * P],
                                 gwts[k][:], start=(k == 0), stop=(k == KD - 1))
            # m1 = rowmax(logits); nm1 = -m1
            m1 = rt_pool.tile([P, 1], F32, tag="m1")
            nc.vector.reduce_max(m1[:], pl[:], axis=mybir.AxisListType.X)
            nm1 = rt_pool.tile([P, 1], F32, tag="nm1")
            nc.vector.tensor_scalar(nm1[:], m1[:], -1.0, None, op0=OP.mult)
            # mask of top-1; m2 = rowmax(logits - BIG*top1mask)
            t1 = rt_pool.tile([P, E], F32, tag="t1")
            nc.vector.tensor_scalar(t1[:], pl[:], m1[:], None, op0=OP.is_ge)
            lm = rt_pool.tile([P, E], F32, tag="lm")
            nc.vector.scalar_tensor_tensor(lm[:], t1[:], -1e30, pl[:],
                                           op0=OP.mult, op1=OP.add)
            m2 = rt_pool.tile([P, 1], F32, tag="m2")
            nc.vector.reduce_max(m2[:], lm[:], axis=mybir.AxisListType.X)
            # el = exp(logits - m1); s = rowsum(el); el1 = rowmax(el)
            el = rt_pool.tile([P, E], F32, tag="el")
            nc.scalar.activation(el[:], pl[:], AF.Exp, bias=nm1[:])
            ssum = rt_pool.tile([P, 1], F32, tag="ssum")
            nc.vector.reduce_sum(ssum[:], el[:], axis=mybir.AxisListType.X)
            el1 = rt_pool.tile([P, 1], F32, tag="el1")
            nc.vector.reduce_max(el1[:], el[:], axis=mybir.AxisListType.X)
            el2 = rt_pool.tile([P, 1], F32, tag="el2")
            nc.scalar.activation(el2[:], m2[:], AF.Exp, bias=nm1[:])
            # denom = el1 + el2 + 1e-8 * s ; rec = 1/denom
            den = rt_pool.tile([P, 1], F32, tag="den")
            nc.vector.tensor_tensor(den[:], el1[:], el2[:], op=OP.add)
            nc.vector.scalar_tensor_tensor(den[:], ssum[:], 1e-8, den[:],
                                           op0=OP.mult, op1=OP.add)
            rec = rt_pool.tile([P, 1], F32, tag="rec")
            nc.vector.reciprocal(rec[:], den[:])
            # cw = (logits >= m2) * el * rec
            m2mask = rt_pool.tile([P, E], F32, tag="m2mask")
            nc.vector.tensor_scalar(m2mask[:], pl[:], m2[:], None, op0=OP.is_ge)
            cwu = rt_pool.tile([P, E], F32, tag="cwu")
            nc.vector.tensor_tensor(cwu[:], m2mask[:], el[:], op=OP.mult)
            cw = rt_pool.tile([P, E], F32, tag="cw")
            nc.vector.tensor_scalar(cw[:], cwu[:], rec[:], None, op0=OP.mult)
            cw_tiles.append(cw)

        # ---- experts: j=0 shared, j=1..8 routed ----
        accs = [acc_pool.tile([P, D], F32, tag="acc", name=f"acc{mt}") for mt in range(MT)]
        for j in range(NE):
            wgut = []
            for k in range(KD):
                w = wgu_pool.tile([P, GU], F32R, tag="wgu")
                nc.sync.dma_start(w[:], wgu[j, k * P:(k + 1) * P, :])
                wgut.append(w)
            wdnt = []
            for k in range(KI):
                w = wdn_pool.tile([P, D], F32R, tag="wdn")
                nc.sync.dma_start(w[:], wdn[j, k * P:(k + 1) * P, :])
                wdnt.append(w)

            # stage 1: A^T[m] (:, n-slice) = silu(G) * U
            ats = [a_pool.tile([P, T], F32R, tag="a", name=f"at{m}") for m in range(KI)]
            for n in range(NT2):
                ns = slice(n * 512, (n + 1) * 512)
                for m in range(KI):
                    pg = ps_g.tile([P, 512], F32, tag="psg")
                    pu = ps_u.tile([P, 512], F32, tag="psu")
                    for k in range(KD):
                        nc.tensor.matmul(pg[:], r(wgut[k][:, m * P:(m + 1) * P]),
                                         r(xts[k][:, ns]),
                                         start=(k == 0), stop=(k == KD - 1))
                    for k in range(KD):
                        nc.tensor.matmul(pu[:], r(wgut[k][:, IP + m * P:IP + (m + 1) * P]),
                                         r(xts[k][:, ns]),
                                         start=(k == 0), stop=(k == KD - 1))
                    st = tmp_pool.tile([P, 512], F32, tag="tmp")
                    nc.scalar.activation(st[:], pg[:], AF.Silu)
                    nc.vector.tensor_tensor(ats[m][:, ns], st[:], pu[:], op=OP.mult)

            # stage 2: OUT[mt, nd] += cw * (A^T.T @ Wdn^T)
            for mt in range(MT):
                for nd in range(ND2):
                    po = ps_o.tile([P, 512], F32, tag="pso")
                    for k in range(KI):
                        nc.tensor.matmul(po[:], r(ats[k][:, mt * P:(mt + 1) * P]),
                                         r(wdnt[k][:, nd * 512:(nd + 1) * 512]),
                                         start=(k == 0), stop=(k == KI - 1))
                    acc_sl = accs[mt][:, nd * 512:(nd + 1) * 512]
                    if j == 0:
                        nc.vector.tensor_copy(acc_sl, po[:])
                    else:
                        nc.vector.scalar_tensor_tensor(
                            acc_sl, po[:], cw_tiles[mt][:, j - 1:j], acc_sl,
                            op0=OP.mult, op1=OP.add)

        # ---- store ----
        for mt in range(MT):
            nc.sync.dma_start(out[mt * P:(mt + 1) * P, :], accs[mt][:])

    nc.compile()
    return nc


_NC_CACHE = None


def _get_nc():
    global _NC_CACHE
    if _NC_CACHE is None:
        _NC_CACHE = build_moe()
    return _NC_CACHE


def _prep_weights(gate_weight, shared_gate_up, shared_down,
                  experts_gate_up, experts_down):
    wgu = np.zeros((NE, D, GU), np.float32)
    wgu[0, :, 0:I] = shared_gate_up[0:I].T
    wgu[0, :, IP:IP + I] = shared_gate_up[I:2 * I].T
    for e in range(E):
        wgu[e + 1, :, 0:I] = experts_gate_up[e, 0:I].T
        wgu[e + 1, :, IP:IP + I] = experts_gate_up[e, I:2 * I].T
    wdn = np.zeros((NE, IP, D), np.float32)
    wdn[0, 0:I, :] = shared_down.T
    for e in range(E):
        wdn[e + 1, 0:I, :] = experts_down[e].T
    gwT = np.ascontiguousarray(gate_weight.T.astype(np.float32))
    return gwT, np.ascontiguousarray(wgu), np.ascontiguousarray(wdn)


def kernel(hidden_states, gate_weight, shared_gate_up, shared_down,
           experts_gate_up, experts_down):
    hidden_states = np.asarray(hidden_states, dtype=np.float32)
    x = hidden_states.reshape(B * S, D)
    gwT, wgu, wdn = _prep_weights(
        np.asarray(gate_weight, np.float32),
        np.asarray(shared_gate_up, np.float32),
        np.asarray(shared_down, np.float32),
        np.asarray(experts_gate_up, np.float32),
        np.asarray(experts_down, np.float32))
    in_maps = []
    for c in range(N_CORES):
        in_maps.append({
            "xT": np.ascontiguousarray(x[c * T:(c + 1) * T].T),
            "gwT": gwT,
            "wgu": wgu,
            "wdn": wdn,
        })
    nc = _get_nc()
    res = run_bass_kernel_spmd(nc, in_maps, core_ids=list(range(N_CORES)))
    out = np.concatenate([res.results[c]["out"] for c in range(N_CORES)], axis=0)
    return out.reshape(B, S, D)
